# revision 40
# baseline (speedup 1.0000x reference)
"""Trainium2 Bass kernel for nn_CnnSnn: Conv1d+BN -> 3-layer LIF SNN.

Strategy (data-parallel over 8 cores, 128 batches each):
  The lax.scan recurrence never couples layers across time through matmuls:
  each LIF layer's membrane depends only on its own input-current sequence.
  So per core we compute
      conv (matmul, K=2048 contraction)  -> encoder LIF scan (elementwise)
      fc1  (matmul, K=512)               -> hidden LIF scan
      fc2  (matmul, K=512, M=2)          -> output LIF scan -> DMA out
  All matmuls in fp32 (PE hi/lo passes, true fp32).  The elementwise chain
  replicates the reference's op-for-op fp32 rounding order so spike
  decisions match the jax fp32 reference.

Layouts (per core, B'=128 split into 8 groups of 16):
  per-m activation tiles [128 part, 3968 free], free col = bg*496 + t*16 + b
  conv rhs x_seg[j, p=(ci_in*16+k'), bg, s, b]  (j: 8 tiles of 128 contraction rows)
  conv out[t] = W1.T @ seg(s=t) + W2.T @ seg(s=t+1)   (kernel split in halves of 16,
  stride 16 makes window t = [seg_t, seg_{t+1}] -- no im2col duplication)
"""

import numpy as np

FS = 128; WIN = 4; C_IN = 64; C_OUT = 512; BATCH = 1024
KW = FS // 4; STRIDE = KW // 2; WS = FS * WIN
T = (WS - KW) // STRIDE + 1          # 31
BN_EPS = 1e-5
NCORES = 8
BPC = BATCH // NCORES                # 128 batches per core
BG = 8                               # batch groups per core
BB = BPC // BG                       # 16 batches per group
M = 4                                # c-tiles of 128
CHUNK = T * BB                       # 496 free columns per (m, bg)
MF = BG * CHUNK                      # 3968 free columns per m-tile

_CACHE = {}


def _build_nc():
    import concourse.bass as bass
    import concourse.mybir as mybir
    from concourse import bacc
    from concourse.tile import TileContext
    from contextlib import ExitStack

    f32 = mybir.dt.float32
    AF = mybir.ActivationFunctionType
    OP = mybir.AluOpType

    # Bacc (not raw Bass): its compile() splits multi-semaphore waits into
    # EventSemaphore instructions -- the TRN2 ISA allows one wait per
    # instruction and walrus rejects more ("Too many sync wait commands").
    nc = bacc.Bacc("TRN2", target_bir_lowering=False, debug=False,
                   num_devices=NCORES)

    # ---- DRAM I/O ----
    x_seg = nc.declare_dram_parameter("x_seg", [8, 128, BG, 32, BB], f32, isOutput=False)
    w1t = nc.declare_dram_parameter("w1t", [8, 128, C_OUT], f32, isOutput=False)
    w2t = nc.declare_dram_parameter("w2t", [8, 128, C_OUT], f32, isOutput=False)
    # fc weights split hi/lo so the matmuls can run as float32r (fp22-read)
    # 2-pass at 1 cycle/row instead of fp32's 4: hi = top 13 mantissa bits
    # (exact in fp22), lo = w - hi (10 significant bits, also exact in fp22),
    # and spike activations are exactly 0.0/1.0, so hi@s + lo@s == w@s with
    # fp32-identical per-product values.
    fc1t = nc.declare_dram_parameter("fc1t", [2, 4, 128, C_OUT], mybir.dt.float32r, isOutput=False)
    fc2t = nc.declare_dram_parameter("fc2t", [2, 4, 128, 2], mybir.dt.float32r, isOutput=False)
    # per-channel params packed into one tensor, arranged [p, field*4 + m]
    # fields: cb, mean, gamma, inv, bnb, f1b, be, bh (8 x [128,4]) then
    # col 32 = f2b (rows 0:2), col 33 = bo (rows 0:2)
    p_all = nc.declare_dram_parameter("p_all", [128, 34], f32, isOutput=False)

    # outputs in SBUF-native layout [c, (g t b)]; host reshapes to [T, B, 2]
    om_o = nc.declare_dram_parameter("om_o", [2, BG, BB], f32, isOutput=True)
    spk_o = nc.declare_dram_parameter("spk_o", [2, MF], f32, isOutput=True)
    mem_o = nc.declare_dram_parameter("mem_o", [2, MF], f32, isOutput=True)

    with TileContext(nc) as tc, ExitStack() as ctx:
        constF = ctx.enter_context(tc.tile_pool(name="constF", bufs=1))
        cc = ctx.enter_context(tc.tile_pool(name="cc", bufs=4))      # cur/spk rolling
        state = ctx.enter_context(tc.tile_pool(name="state", bufs=6))
        vpool = ctx.enter_context(tc.tile_pool(name="vpool", bufs=8))
        psum = ctx.enter_context(tc.tile_pool(name="psum", bufs=6, space="PSUM"))
        psum2 = ctx.enter_context(tc.tile_pool(name="psum2", bufs=2, space="PSUM"))
        sepool = ctx.enter_context(tc.tile_pool(name="sepool", bufs=8))

        # ---- constants ----
        f32r = mybir.dt.float32r  # spikes/split-weights are exact in fp22
        fc1s = []   # [hl][ke]
        for hl in range(2):
            row = []
            for ke in range(4):
                tl = constF.tile([128, C_OUT], f32r, tag=f"fc1s{hl}{ke}")
                nc.sync.dma_start(tl[:], fc1t[hl, ke])
                row.append(tl)
            fc1s.append(row)
        fc2s = []
        for hl in range(2):
            row = []
            for kh in range(4):
                tl = constF.tile([128, 2], f32r, tag=f"fc2s{hl}{kh}")
                nc.sync.dma_start(tl[:], fc2t[hl, kh])
                row.append(tl)
            fc2s.append(row)

        pk = constF.tile([128, 34], f32, tag="pk")
        nc.sync.dma_start(pk[:], p_all[:, :])
        cb, mn, gm, iv = pk[:, 0:4], pk[:, 4:8], pk[:, 8:12], pk[:, 12:16]
        bb_, f1b, be_s, bh_s = pk[:, 16:20], pk[:, 20:24], pk[:, 24:28], pk[:, 28:32]
        f2b, bo_s = pk[0:2, 32:33], pk[0:2, 33:34]

        zero = constF.tile([128, 128], f32, tag="zero")
        nc.vector.memset(zero[:], 0.0)
        zv = zero.rearrange("p (g b) -> p g b", b=BB)

        def lif_scan(cur_t, spk_t, beta_ap, g, npart=128):
            # cur_t/spk_t: [p, g*CHUNK] tiles; scan over t, all (bg, b) cols.
            # op1/op2 on DVE, the mem update on GPSIMD (parallel engine).
            cur_v = cur_t.rearrange("p (g t b) -> p g t b", g=g, t=T)
            spk_v = spk_t.rearrange("p (g t b) -> p g t b", g=g, t=T)
            mem = state.tile([npart, g * BB], f32, tag=f"mem{npart}")
            nc.vector.memset(mem[:], 0.0)
            memv = mem.rearrange("p (g b) -> p g b", b=BB)
            prev = zv[:npart, :g]
            for t in range(T):
                v = vpool.tile([npart, g * BB], f32, tag=f"v{npart}")
                vv = v.rearrange("p (g b) -> p g b", b=BB)
                nc.vector.scalar_tensor_tensor(
                    vv, memv, beta_ap, cur_v[:, :, t, :], OP.mult, OP.add)
                nc.vector.scalar_tensor_tensor(
                    spk_v[:, :, t, :], vv, 1.0, prev, OP.subtract, OP.is_gt)
                nc.gpsimd.tensor_tensor(memv, vv, prev, OP.subtract)
                prev = spk_v[:, :, t, :]

        # ---- pipelined phases over two batch halves ----
        # Emission order convA convB fc1A fc1B fc2A fc2B keeps the PE stream
        # dense: each half's LIF scans (DVE/GPSIMD) run in the shadow of the
        # other half's matmuls.
        wpool = ctx.enter_context(tc.tile_pool(name="wpool", bufs=2))
        xpool = ctx.enter_context(tc.tile_pool(name="xpool", bufs=2))
        l3pool = ctx.enter_context(tc.tile_pool(name="l3", bufs=1))
        cur2 = l3pool.tile([2, MF], f32, tag="cur2")
        spk3 = cur2   # output spikes overwrite consumed currents in place
        mem3 = l3pool.tile([2, MF], f32, tag="mem3")
        HG = BG // 2                      # batch groups per half
        HF = HG * CHUNK                   # free columns per half per m

        ses = {}
        def conv_half(h):
            for m in range(M):
                w1s, w2s = [], []
                for j in range(8):
                    tl = wpool.tile([128, 128], f32, tag=f"w1m{j}")
                    nc.sync.dma_start(tl[:], w1t[j, :, m * 128:(m + 1) * 128])
                    w1s.append(tl)
                for j in range(8):
                    tl = wpool.tile([128, 128], f32, tag=f"w2m{j}")
                    nc.sync.dma_start(tl[:], w2t[j, :, m * 128:(m + 1) * 128])
                    w2s.append(tl)
                conv_cur = cc.tile([128, HF], f32, tag="cc")
                for bgl in range(HG):
                    bg = h * HG + bgl
                    xt = xpool.tile([128, 8 * 32 * BB], f32, tag="xt")
                    xv = xt.rearrange("p (j s b) -> p j s b", j=8, s=32)
                    for j in range(8):
                        nc.sync.dma_start(xv[:, j, :, :], x_seg[j, :, bg])
                    ps = psum.tile([128, CHUNK], f32, tag="ps")
                    pv = ps.rearrange("p (t b) -> p t b", b=BB)
                    for j in range(8):
                        nc.tensor.matmul(pv, w1s[j][:], xv[:, j, 0:T, :],
                                         start=(j == 0), stop=False)
                    for j in range(8):
                        nc.tensor.matmul(pv, w2s[j][:], xv[:, j, 1:T + 1, :],
                                         start=False, stop=(j == 7))
                    dst = conv_cur[:, bgl * CHUNK:(bgl + 1) * CHUNK]
                    # replicate reference rounding: (+b), (-mean)*gamma, (*inv)+beta
                    nc.scalar.activation(dst, ps[:], AF.Identity,
                                         bias=cb[:, m:m + 1])
                    nc.vector.tensor_scalar(dst, dst, mn[:, m:m + 1],
                                            gm[:, m:m + 1], OP.subtract, OP.mult)
                    nc.vector.tensor_scalar(dst, dst, iv[:, m:m + 1],
                                            bb_[:, m:m + 1], OP.mult, OP.add)
                se = sepool.tile([128, HF], f32r, tag="se")
                lif_scan(conv_cur, se, be_s[:, m:m + 1], HG)
                ses[(h, m)] = se

        def fc_half(h):
            for m in range(M):
                cur1 = cc.tile([128, HF], f32, tag="cc")
                for bgl in range(HG):
                    ps = psum.tile([128, CHUNK], f32, tag="ps")
                    for i, (hl, ke) in enumerate([(a, k) for a in range(2)
                                                  for k in range(4)]):
                        nc.tensor.matmul(
                            ps[:],
                            fc1s[hl][ke][:, m * 128:(m + 1) * 128],
                            ses[(h, ke)][:, bgl * CHUNK:(bgl + 1) * CHUNK],
                            start=(i == 0), stop=(i == 7))
                    nc.scalar.activation(cur1[:, bgl * CHUNK:(bgl + 1) * CHUNK],
                                         ps[:], AF.Identity, bias=f1b[:, m:m + 1])
                sh = cc.tile([128, HF], f32r, tag="cc")
                lif_scan(cur1, sh, bh_s[:, m:m + 1], HG)
                # fc2 partial for this m (accumulate into cur2; same rounding
                # order as sequential PSUM accumulation)
                for bgl in range(HG):
                    bg = h * HG + bgl
                    ps2 = psum2.tile([2, CHUNK], f32, tag="ps2")
                    for hl in range(2):
                        nc.tensor.matmul(
                            ps2[:], fc2s[hl][m][:],
                            sh[:, bgl * CHUNK:(bgl + 1) * CHUNK],
                            start=(hl == 0), stop=(hl == 1))
                    dst2 = cur2[:, bg * CHUNK:(bg + 1) * CHUNK]
                    if m == 0:
                        nc.scalar.copy(dst2, ps2[:])
                    else:
                        nc.vector.tensor_tensor(dst2, dst2, ps2[:], OP.add)

        def out_half(h):
            sl = slice(h * HG * CHUNK, (h + 1) * HG * CHUNK)
            nc.vector.tensor_scalar(cur2[:, sl], cur2[:, sl], f2b, None, OP.add)
            c2v = cur2[:, sl].rearrange("p (g t b) -> p g t b", g=HG, t=T)
            s3v = spk3[:, sl].rearrange("p (g t b) -> p g t b", g=HG, t=T)
            m3v = mem3[:, sl].rearrange("p (g t b) -> p g t b", g=HG, t=T)
            prev = zv[:2, :HG]
            prev_mem = zv[:2, :HG]
            for t in range(T):
                v3 = vpool.tile([2, HG * BB], f32, tag="v3")
                v3v = v3.rearrange("p (g b) -> p g b", b=BB)
                nc.vector.scalar_tensor_tensor(
                    v3v, prev_mem, bo_s[:, 0:1], c2v[:, :, t, :], OP.mult, OP.add)
                nc.vector.scalar_tensor_tensor(
                    s3v[:, :, t, :], v3v, 1.0, prev, OP.subtract, OP.is_gt)
                nc.gpsimd.tensor_tensor(m3v[:, :, t, :], v3v, prev, OP.subtract)
                prev = s3v[:, :, t, :]
                prev_mem = m3v[:, :, t, :]

        conv_half(0)
        conv_half(1)
        fc_half(0)
        fc_half(1)
        out_half(0)
        out_half(1)

        # ---- outputs ----
        m3full = mem3.rearrange("p (g t b) -> p g t b", g=BG, t=T)
        nc.sync.dma_start(spk_o[:, :], spk3[:])
        nc.sync.dma_start(mem_o[:, :], mem3[:])
        nc.sync.dma_start(om_o[:, :, :], m3full[:, :, T - 1, :])

    nc.compile()
    return nc


def _prep_host(inputs):
    """Host-side re-layout of inputs into the kernel's DRAM formats."""
    f32 = np.float32
    x = np.asarray(inputs["x"], f32)
    conv_w = np.asarray(inputs["conv_w"], f32)

    # x_seg[core][j, p=(ci_in,k'), bg, s, b] = x[core*128 + bg*16+b, 8j+ci_in, 16s+k']
    xs = x.reshape(NCORES, BG, BB, C_IN, 32, 16)
    xt = xs.transpose(0, 3, 5, 1, 4, 2)          # core, ci, k', bg, s, b
    xseg = np.ascontiguousarray(xt).reshape(NCORES, 8, 8 * 16, BG, 32, BB)
    # ci -> (j, ci_in) split via the reshape: ci = 8j + ci_in, p = ci_in*16 + k'

    w1 = conv_w[:, :, :16].transpose(1, 2, 0)    # ci, k', co
    w2 = conv_w[:, :, 16:].transpose(1, 2, 0)
    w1t = np.ascontiguousarray(w1).reshape(8, 128, C_OUT)
    w2t = np.ascontiguousarray(w2).reshape(8, 128, C_OUT)

    def split_f32r(a):
        """w = hi + lo with both pieces exactly representable in fp22."""
        hi = (a.view(np.uint32) & np.uint32(0xFFFFF000)).view(f32)
        lo = (a - hi).astype(f32)
        return np.stack([hi, lo])

    fc1t = split_f32r(
        np.ascontiguousarray(np.asarray(inputs["fc1_w"], f32).T).reshape(4, 128, C_OUT))
    fc2t = split_f32r(
        np.ascontiguousarray(np.asarray(inputs["fc2_w"], f32).T).reshape(4, 128, 2))

    def pm(a):   # [512] -> [p, m]
        return np.ascontiguousarray(np.asarray(a, f32).reshape(4, 128).T)

    bn_var = np.asarray(inputs["bn_var"], f32)
    inv32 = _rsqrt_f32(bn_var + f32(BN_EPS))

    p_all = np.zeros((128, 34), f32)
    fields = [inputs["conv_b"], inputs["bn_mean"], inputs["bn_gamma"], inv32,
              inputs["bn_beta"], inputs["fc1_b"],
              np.clip(np.asarray(inputs["beta_enc"], f32), 0.0, 1.0),
              np.clip(np.asarray(inputs["beta_hid"], f32), 0.0, 1.0)]
    for i, a in enumerate(fields):
        p_all[:, 4 * i:4 * i + 4] = pm(a)
    p_all[0:2, 32] = np.asarray(inputs["fc2_b"], f32)
    p_all[0:2, 33] = np.clip(np.asarray(inputs["beta_out"], f32), 0.0, 1.0)
    shared = {"w1t": w1t, "w2t": w2t, "fc1t": fc1t, "fc2t": fc2t, "p_all": p_all}
    return xseg, shared


def _rsqrt_f32(a):
    """Bitwise-match jax.lax.rsqrt on f32 inputs (reference uses it for BN)."""
    try:
        import jax
        import jax.numpy as jnp
        with jax.default_device(jax.devices("cpu")[0]):
            return np.asarray(jax.lax.rsqrt(jnp.asarray(a, jnp.float32)))
    except Exception:
        return (1.0 / np.sqrt(np.asarray(a, np.float64))).astype(np.float32)


def kernel(**inputs):
    from concourse.bass_utils import run_bass_kernel_spmd

    if "nc" not in _CACHE:
        _CACHE["nc"] = _build_nc()
    nc = _CACHE["nc"]

    xseg, shared = _prep_host(inputs)
    in_maps = [{"x_seg": xseg[c], **shared} for c in range(NCORES)]
    res = run_bass_kernel_spmd(nc, in_maps, list(range(NCORES)))
    outs = res.results

    def fix_rec(a):   # [2, MF] -> [T, BPC, 2]
        return np.ascontiguousarray(
            a.reshape(2, BG, T, BB).transpose(2, 1, 3, 0).reshape(T, BPC, 2))

    def fix_om(a):    # [2, BG, BB] -> [BPC, 2]
        return np.ascontiguousarray(
            a.reshape(2, BPC).T)

    om = np.concatenate([fix_om(outs[c]["om_o"]) for c in range(NCORES)], axis=0)
    spk = np.concatenate([fix_rec(outs[c]["spk_o"]) for c in range(NCORES)], axis=1)
    mem = np.concatenate([fix_rec(outs[c]["mem_o"]) for c in range(NCORES)], axis=1)
    return om, spk, mem
